# revision 33
# baseline (speedup 1.0000x reference)
"""Trainium2 Bass kernel for nn_ActionDecoder (moe_routing) — fp8 DoubleRow.

Data-parallel across 8 NeuronCores: batch 4096 -> 512 per core, weights
replicated. Host deals samples to cores balanced per command value and sorts
each core's 512 samples by command, so each head's samples occupy a fixed
column segment [a_h, e_h) (identical offsets on all cores -> one SPMD graph).

All GEMMs run in fp8-e4m3 DoubleRow perf mode with f32 PSUM accumulation;
rel err ~1.7e-3 vs the f32 reference. Power-of-two scales keep descale
exact: x as-is, W_fc*32 -> hp stores 32*h' (range <= 178 < 240 TRN-fp8
max), W1*32 with b1 riding an augmented constant-32 row of the ego chunk,
z1 stores 16*relu(.), W2*16 -> z2 = psum/256 + b2.

Trace-driven notes (HW, per-core):
- Steady trunk DR matmul = 216 ns (512 cols @ 2.4 GHz, 157 TF/s fp8 peak).
  Pure trunk compute = 57 us; everything else is overhead to shave.
- DMA: only gpsimd (SWDGE) + scalar/sync (HWDGE) rings exist. Two fast
  rings sustain ~280-300 GB/s combined on 4KB lines. Small-packet loads
  (1KB w1 trickle on sync in the old layout) steal DMA-engine slots from
  the fast rings -> mid-trunk stalls. Now sync carries only ~30KB of
  consts; w1 rides the scalar ring's FIFO tail after all W quads.
- PE clock starts at half rate and ramps only under sustained activity:
  junk warmup matmuls run from body start until real data lands.
- Head phase: per head one merged z1 psum [128,2,C] (one drain), one
  DoubleRow z2 matmul, mask-multiplied into one of two disjoint
  accumulators (even/odd heads), epilogue in 3 pipelined column slices.
"""

import numpy as np
import ml_dtypes

B = 4096
EMBED = 8192
U0 = 1024
U1 = 256
EGO = 3
H = 6
NCORES = 8
BC = B // NCORES          # 512 batch per core
KP = EMBED // 256         # 32 trunk k-pairs (DoubleRow: 256 k per matmul)
KO = EMBED // 1024        # 8 trunk k-octs (one x DMA each, 4KB lines)
NCH = U0 // 128           # 8 trunk n-chunks
NWARM = 7                 # 512-col junk matmuls to ramp the PE clock
MEAN_SCALE = 5.0
INIT_STD = 5.0
MIN_STD = 1e-4
SH = 32.0                 # scale on W_fc / hp
SW1 = 32.0                # scale on W1 (+bias row)
SZ = 16.0                 # scale on stored z1
SW2 = 16.0                # scale on W2

FP8 = ml_dtypes.float8_e4m3   # TRN fp8_e4m3 semantics (max 240)


def _build_graph(seg):
    """seg: list of (a_h, C_h) column segments per head, identical on all cores."""
    import concourse.mybir as mybir
    import concourse.tile as tile
    from concourse import bacc

    dt = mybir.dt
    AF = mybir.ActivationFunctionType
    DR = mybir.MatmulPerfMode.DoubleRow

    nc = bacc.Bacc("TRN2", target_bir_lowering=False, debug=False)

    xp = nc.dram_tensor("xp", [KO * 128, 8, BC], dt.float8e4, kind="ExternalInput")
    wp = nc.dram_tensor("wp", [KP * 64, 4, U0], dt.float8e4, kind="ExternalInput")
    bfcT = nc.dram_tensor("bfcT", [128, NCH], dt.float32, kind="ExternalInput")
    egoT = nc.dram_tensor("egoT", [EGO + 1, BC], dt.float8e4, kind="ExternalInput")
    w1pd = nc.dram_tensor("w1pd", [128, H * 4 * 2, 256], dt.float8e4, kind="ExternalInput")
    w1ed = nc.dram_tensor("w1ed", [128, H * 256], dt.float8e4, kind="ExternalInput")
    w2pd = nc.dram_tensor("w2pd", [128, H * 2, 4], dt.float8e4, kind="ExternalInput")
    b2m = nc.dram_tensor("b2m", [4, H], dt.float32, kind="ExternalInput")
    hid = nc.dram_tensor("hid", [4, BC], dt.float32, kind="ExternalInput")
    epi = nc.dram_tensor("epi", [4, 2], dt.float32, kind="ExternalInput")
    out_d = nc.dram_tensor("out", [4, BC], dt.float32, kind="ExternalOutput")

    with tile.TileContext(nc) as tc:
        with (
            tc.tile_pool(name="const", bufs=1) as const,
            tc.tile_pool(name="xk", bufs=8) as xpool,
            tc.tile_pool(name="wk", bufs=16) as wpool,
            tc.tile_pool(name="hp", bufs=1) as hpool,
            tc.tile_pool(name="z1", bufs=1) as zpool,
            tc.tile_pool(name="ps", bufs=8, space="PSUM") as psum,
        ):
            # h' ego chunk: rows 0-2 = 32*ego, row 3 = 32 (bias row), rest 0.
            # memset first on vector so the warmup matmuls can start at body
            # entry, while the DMA rings ramp up.
            hpe = hpool.tile([128, BC], dt.float8e4, tag="hpe")
            nc.vector.memset(hpe[:], 0.0)

            ps_h = [
                psum.tile([128, BC], dt.float32, tag="ps", name=f"ps_h{n}")
                for n in range(NCH)
            ]

            # PE clock warmup: long junk matmuls keep the PE activity monitor
            # busy from body start so the clock ramps to 2.4 GHz during the
            # DMA fill instead of midway through the trunk.
            for i in range(NWARM):
                nc.tensor.matmul(
                    ps_h[NCH - 1][:],
                    hpe[:, 0:128],
                    hpe[:, 0:BC],
                    start=True,
                    stop=True,
                )

            # oct0's x j0/j1 ride the sync HWDGE ring ahead of the consts:
            # sync starts fastest and is otherwise idle, so the first trunk
            # matmul's inputs (64KB x + 128KB W) land earliest.
            xk0 = xpool.tile([128, 8, BC], dt.float8e4, tag="xk", name="xo0")
            nc.sync.dma_start(out=xk0[:, 0, :], in_=xp[0:128, 0, :])
            nc.sync.dma_start(out=xk0[:, 1, :], in_=xp[0:128, 1, :])

            # consts on the sync ring: ~30KB total, needed only late. All
            # slots padded to 64B multiples: SBUF packing aligns only to
            # 32B, and a 32B-offset matmul operand halves the PE fetch rate.
            bfc_t = const.tile([128, NCH], dt.float32, tag="bfc",
                               padded_shape=[128, 16])
            nc.sync.dma_start(out=bfc_t[:], in_=bfcT[:])
            b2_t = const.tile([4, H], dt.float32, tag="b2",
                              padded_shape=[4, 16])
            nc.sync.dma_start(out=b2_t[:], in_=b2m[:])
            hid_t = const.tile([4, BC], dt.float32, tag="hid")
            nc.sync.dma_start(out=hid_t[:], in_=hid[:])
            w2_t = const.tile([128, H * 2, 4], dt.float8e4, tag="w2",
                              padded_shape=[128, 16, 4])
            nc.sync.dma_start(out=w2_t[:], in_=w2pd[:])
            epi_t = const.tile([4, 2], dt.float32, tag="epi",
                               padded_shape=[4, 16])
            nc.sync.dma_start(out=epi_t[:], in_=epi[:])
            nc.sync.dma_start(out=hpe[0 : EGO + 1, :], in_=egoT[:])

            w1p_t = const.tile([128, H * 4 * 2, 256], dt.float8e4, tag="w1p")
            w1e_t = const.tile([128, H * 256], dt.float8e4, tag="w1e")

            hp_pair = [
                hpool.tile([128, 2, BC], dt.float8e4, tag=f"hpp{t}", name=f"hpp{t}")
                for t in range(4)
            ]

            # masked-z2 accumulators: even heads write zA slices, odd heads
            # zB; segments of same-parity heads are disjoint, so each is a
            # plain masked write and the epilogue adds the two.
            zA = const.tile([4, BC], dt.float32, tag="zA")
            zB = const.tile([4, BC], dt.float32, tag="zB")
            nc.vector.memset(zA[:], 0.0)
            nc.vector.memset(zB[:], 0.0)

            # per-head one-hot masks: emitted up front so the vector engine
            # computes them as soon as hid lands (~14us), not at head time
            mks = []
            for h in range(H):
                a, C = seg[h]
                Cp = (C + 15) // 16 * 16
                mk = const.tile([4, C], dt.float32, tag=f"mk_{h}", name=f"mk_{h}",
                                padded_shape=[4, Cp])
                nc.vector.tensor_scalar(
                    mk[:], hid_t[:, a : a + C], float(h + 1), None,
                    mybir.AluOpType.is_equal,
                )
                mks.append(mk)

            # head-phase z1 state, created lazily as the weave needs it
            pzs = [None] * H
            z1ps = [None] * H

            def _z1_alloc(h):
                a, C = seg[h]
                Cp = (C + 31) // 32 * 32
                pzs[h] = psum.tile(
                    [128, 2, C], dt.float32, tag="ps", name=f"pz_{h}"
                )
                z1ps[h] = zpool.tile(
                    [128, 2, C], dt.float8e4, tag=f"z1_{h}", name=f"z1_{h}",
                    padded_shape=[128, 2, Cp],
                )

            def _z1_ego(h):
                a, C = seg[h]
                for m in range(2):
                    nc.tensor.matmul(
                        pzs[h][:, m, :],
                        w1e_t[:, h * 256 + m * 128 : h * 256 + (m + 1) * 128],
                        hpe[:, a : a + C],
                        start=True,
                        stop=False,
                    )

            def _z1_pair(h, t):
                a, C = seg[h]
                g = (h * 4 + t) * 2
                for m in range(2):
                    nc.tensor.matmul(
                        pzs[h][:, m, :],
                        w1p_t[:, g : g + 2, m * 128 : (m + 1) * 128],
                        hp_pair[t][:, :, a : a + C],
                        start=False,
                        stop=(t == 3),
                        perf_mode=DR,
                    )

            # trunk: hp = relu((32Wfc).T @ x + 32 b_fc), DoubleRow over
            # k-pairs; x streams in k-octs and W in k-quads (512KB DMAs with
            # 4KB partition lines; the rings are packet-rate-bound).
            for ob in range(KO):
                first_block = ob == 0
                last_block = ob == KO - 1
                if first_block:
                    xk = xk0
                else:
                    xk = xpool.tile(
                        [128, 8, BC], dt.float8e4, tag="xk", name=f"xo{ob}"
                    )
                wq0 = wpool.tile([128, 4, U0], dt.float8e4, tag="wk", name=f"wq{2 * ob}")
                wq1 = wpool.tile([128, 4, U0], dt.float8e4, tag="wk", name=f"wq{2 * ob + 1}")
                r = slice(ob * 128, (ob + 1) * 128)
                r0 = slice(2 * ob * 128, (2 * ob + 1) * 128)
                r1 = slice((2 * ob + 1) * 128, (2 * ob + 2) * 128)
                if first_block:
                    # ramp: x j0/j1 already posted on sync; W pair-sliced on
                    # scalar, the rest of x + wq1 on gpsimd. Keeping oct0's W
                    # on the fast rings matters doubly: sparse early matmul
                    # activity delays the PE clock ramp and lengthens the
                    # governor's corrective throttle window.
                    nc.scalar.dma_start(out=wq0[:, 0, :], in_=wp[r0, 0, :])
                    nc.scalar.dma_start(out=wq0[:, 1, :], in_=wp[r0, 1, :])
                    nc.gpsimd.dma_start(out=xk[:, 2:4, :], in_=xp[r, 2:4, :])
                    nc.scalar.dma_start(out=wq0[:, 2:4, :], in_=wp[r0, 2:4, :])
                    nc.gpsimd.dma_start(out=xk[:, 4:8, :], in_=xp[r, 4:8, :])
                    nc.gpsimd.dma_start(out=wq1[:, 0:2, :], in_=wp[r1, 0:2, :])
                    nc.gpsimd.dma_start(out=wq1[:, 2:4, :], in_=wp[r1, 2:4, :])
                else:
                    # x + wq0 alternate over the two fast rings; sync (the
                    # slowest ring, ~0.5MB/10us) carries only the late wq1
                    # quads of octs 3/5/7, each due well after it can land.
                    qa, qb_ = (nc.scalar, nc.gpsimd) if ob % 2 else (nc.gpsimd, nc.scalar)
                    qa.dma_start(out=xk[:], in_=xp[r])
                    qb_.dma_start(out=wq0[:], in_=wp[r0])
                    if ob % 2 and ob > 1:
                        nc.sync.dma_start(out=wq1[:], in_=wp[r1])
                    else:
                        qa.dma_start(out=wq1[:], in_=wp[r1])
                if last_block:
                    # w1 rides the scalar ring's FIFO tail: delivered after
                    # every W quad (~50us), needed by the z1 weave (~70us).
                    nc.scalar.dma_start(out=w1p_t[:, :24, :], in_=w1pd[:, :24, :])
                    nc.scalar.dma_start(out=w1p_t[:, 24:, :], in_=w1pd[:, 24:, :])
                    nc.scalar.dma_start(out=w1e_t[:], in_=w1ed[:])
                if first_block:
                    # j-major: a full row of 8 n-chunk matmuls per j-slice,
                    # so each row's ~3.5us of PE work covers the DMA ramp
                    # latency of the next slice and the PE never stalls on a
                    # W slice that hasn't landed yet.
                    for j in range(2):
                        for n in range(NCH):
                            nc.tensor.matmul(
                                ps_h[n][:],
                                wq0[:, j, n * 128 : (n + 1) * 128],
                                xk[:, j, :],
                                start=(j == 0),
                                stop=False,
                            )
                    for pp in range(1, 4):
                        wq, u = (wq0, 1) if pp < 2 else (wq1, pp - 2)
                        for n in range(NCH):
                            nc.tensor.matmul(
                                ps_h[n][:],
                                wq[:, 2 * u : 2 * u + 2, n * 128 : (n + 1) * 128],
                                xk[:, 2 * pp : 2 * pp + 2, :],
                                start=False,
                                stop=False,
                                perf_mode=DR,
                            )
                    continue
                for n in range(NCH):
                    if True:
                        for pp in range(4):
                            wq, u = (wq0, pp) if pp < 2 else (wq1, pp - 2)
                            nc.tensor.matmul(
                                ps_h[n][:],
                                wq[:, 2 * u : 2 * u + 2, n * 128 : (n + 1) * 128],
                                xk[:, 2 * pp : 2 * pp + 2, :],
                                start=False,
                                stop=(last_block and pp == 3),
                                perf_mode=DR,
                            )
                    if last_block:
                        # n-chunk complete: drain to fp8 SBUF (relu + bias).
                        # n7 splits across both engines so the final pair is
                        # ready ~0.45us after the last trunk matmul.
                        tgt = hp_pair[n // 2][:, n % 2, :]
                        if n == 7:
                            nc.scalar.activation(
                                tgt[:, 0:256], ps_h[n][:, 0:256], AF.Relu,
                                bias=bfc_t[:, n : n + 1], scale=1.0,
                            )
                            nc.vector.tensor_scalar(
                                tgt[:, 256:BC], ps_h[n][:, 256:BC],
                                bfc_t[:, n : n + 1], 0.0,
                                mybir.AluOpType.add, mybir.AluOpType.max,
                            )
                        elif n % 2 == 0:
                            nc.scalar.activation(
                                tgt, ps_h[n][:], AF.Relu,
                                bias=bfc_t[:, n : n + 1], scale=1.0,
                            )
                        else:
                            nc.vector.tensor_scalar(
                                tgt, ps_h[n][:], bfc_t[:, n : n + 1], 0.0,
                                mybir.AluOpType.add, mybir.AluOpType.max,
                            )
                        # weave head z1 matmuls into the last oct, each batch
                        # one n-chunk after its pair drains so the drains
                        # finish under trunk matmul cover and the PE never
                        # stalls; the z1 work runs while trunk activity keeps
                        # the PE clock at full rate.
                        if n == 2:
                            for h in (0, 1):
                                _z1_alloc(h)
                                _z1_ego(h)
                                _z1_pair(h, 0)
                        elif n == 4:
                            for h in (2, 3):
                                _z1_alloc(h)
                                _z1_ego(h)
                                _z1_pair(h, 0)
                            for h in range(4):
                                _z1_pair(h, 1)
                        elif n == 6:
                            for h in (4, 5):
                                _z1_alloc(h)
                                _z1_ego(h)
                                _z1_pair(h, 0)
                                _z1_pair(h, 1)
                            for h in range(H):
                                _z1_pair(h, 2)
                        elif n == 7:
                            for h in range(H):
                                _z1_pair(h, 3)

            s4 = const.tile([4, BC], dt.float32, tag="s4")
            u4 = const.tile([4, BC], dt.float32, tag="u4")
            a4 = const.tile([4, BC], dt.float32, tag="a4")
            t4 = const.tile([4, BC], dt.float32, tag="t4")
            m4 = const.tile([4, BC], dt.float32, tag="m4")

            def _emit_epilogue(sl, mean_ring=None):
                # mean rows 0-1: 5*tanh(z/5); std rows 2-3: softplus(z+b) ~=
                # z + exp(-z) (z ~ 5 here). ACT/DVE can't start at a nonzero
                # partition, so both row pairs run over all 4 partitions with
                # per-row scales (junk in the other pair); the output DMAs
                # are row-disjoint and ride two rings to halve post latency.
                nc.vector.tensor_tensor(
                    s4[:, sl], zA[:, sl], zB[:, sl], mybir.AluOpType.add
                )
                nc.scalar.activation(
                    u4[:, sl], s4[:, sl], AF.Exp, scale=epi_t[:, 1:2]
                )
                nc.scalar.activation(
                    t4[:, sl], s4[:, sl], AF.Tanh, scale=epi_t[:, 0:1]
                )
                nc.vector.tensor_tensor(
                    a4[:, sl], s4[:, sl], u4[:, sl], mybir.AluOpType.add
                )
                nc.vector.tensor_scalar_mul(m4[:, sl], t4[:, sl], MEAN_SCALE)
                nc.sync.dma_start(out=out_d[2:4, sl], in_=a4[2:4, sl])
                (mean_ring or nc.gpsimd).dma_start(
                    out=out_d[0:2, sl], in_=m4[0:2, sl]
                )

            # z1 drains first (all heads' z1 matmuls already flowed during
            # the last oct), then z2 + masked write per head; epilogue in 3
            # pipelined column slices.
            for h in range(H):
                if h % 2 == 0:
                    nc.scalar.activation(
                        z1ps[h][:], pzs[h][:], AF.Relu, scale=SZ / (SH * SW1)
                    )
                else:
                    nc.vector.tensor_scalar(
                        z1ps[h][:], pzs[h][:], SZ / (SH * SW1), 0.0,
                        mybir.AluOpType.mult, mybir.AluOpType.max,
                    )
            slice_bounds = [seg[3][0], seg[5][0], BC]
            slice_after = {2: 0, 4: 1, 5: 2}
            fin = 0
            for h in range(H):
                a, C = seg[h]
                # z2: narrow stationary [128,2,4] fails the DR fp8 LDW ISA
                # restriction, so two plain matmuls over the m-chunks.
                pz2 = psum.tile([4, C], dt.float32, tag="ps", name=f"pz2_{h}")
                for m in range(2):
                    nc.tensor.matmul(
                        pz2[:],
                        w2_t[:, h * 2 + m, :],
                        z1ps[h][:, m, :],
                        start=(m == 0),
                        stop=(m == 1),
                    )
                z2s = const.tile([4, C], dt.float32, tag=f"z2s_{h}", name=f"z2s_{h}",
                                 padded_shape=[4, (C + 15) // 16 * 16])
                if h % 2 == 0:
                    nc.vector.tensor_scalar(
                        z2s[:], pz2[:], 1.0 / (SZ * SW2), b2_t[:, h : h + 1],
                        mybir.AluOpType.mult, mybir.AluOpType.add,
                    )
                else:
                    nc.scalar.activation(
                        z2s[:], pz2[:], AF.Identity,
                        bias=b2_t[:, h : h + 1], scale=1.0 / (SZ * SW2),
                    )
                zX = zA if h % 2 == 0 else zB
                nc.vector.tensor_tensor(
                    zX[:, a : a + C], z2s[:], mks[h][:], mybir.AluOpType.mult
                )
                if h in slice_after:
                    si = slice_after[h]
                    end = slice_bounds[si]
                    if end > fin:
                        # the last slice's mean post rides the scalar ring,
                        # idle once its activations are done
                        _emit_epilogue(
                            slice(fin, end),
                            mean_ring=nc.scalar if si == 2 else None,
                        )
                        fin = end

    nc.compile()
    return nc


def _route(command):
    """Deal samples to cores balanced per head; sort each core by head.

    Returns (perms, seg): perms[c] = global sample indices for core c in
    column order; seg[h] = (a_h, C_h) identical across cores, covering every
    head-h sample's column on every core.
    """
    command = np.asarray(command, dtype=np.int32)
    glob_counts = np.array([(command == h + 1).sum() for h in range(H)], np.int64)
    shares = np.tile(glob_counts // NCORES, (NCORES, 1))
    ptr = 0
    for h in range(H):
        for _ in range(int(glob_counts[h] % NCORES)):
            shares[ptr % NCORES, h] += 1
            ptr += 1
    assert (shares.sum(axis=1) == BC).all()
    percore = [[] for _ in range(NCORES)]
    counts = np.zeros((NCORES, H), np.int64)
    for h in range(H):
        idx = np.nonzero(command == h + 1)[0]
        off = 0
        for c in range(NCORES):
            share = idx[off : off + shares[c, h]]
            off += shares[c, h]
            percore[c].append(share)
            counts[c, h] = len(share)
    perms = [np.concatenate(percore[c]) for c in range(NCORES)]
    starts = np.zeros((NCORES, H), np.int64)
    starts[:, 1:] = np.cumsum(counts, axis=1)[:, :-1]
    ends = starts + counts
    seg = []
    for h in range(H):
        a = int(starts[:, h].min())
        e = int(ends[:, h].max())
        if e <= a:  # head empty on every core: 1 masked-off dummy column
            a = min(a, BC - 1)
            e = a + 1
        seg.append((a, e - a))
    for c in range(NCORES):
        for h in range(H):
            a, C = seg[h]
            assert starts[c, h] >= a and ends[c, h] <= a + C
        assert len(perms[c]) == BC
    # same-parity heads must have disjoint segments (zA/zB write scheme)
    for h in range(H - 2):
        assert seg[h][0] + seg[h][1] <= seg[h + 2][0]
    return perms, seg


def _q8(a):
    return np.clip(a, -240.0, 240.0).astype(FP8)


def _prep_inputs(x, command, ego_state, W_fc, b_fc, W1, b1, W2, b2, perms):
    """Host-side shard + layout prep. Returns in_maps for 8 cores."""
    x = np.asarray(x, dtype=np.float32)
    command = np.asarray(command, dtype=np.int32)
    ego_state = np.asarray(ego_state, dtype=np.float32)

    xq = _q8(x)  # [B, EMBED] fp8
    # W_fc pairs: [kp*128+p, j, n] = 32*Wfc[kp*256 + j*128 + p, n]
    wp_host = np.ascontiguousarray(
        _q8(SH * np.asarray(W_fc, np.float32))
        .reshape(KP // 2, 4, 128, U0)
        .transpose(0, 2, 1, 3)
        .reshape(KP * 64, 4, U0)
    )
    bfcT = np.ascontiguousarray(
        (SH * np.asarray(b_fc, np.float32)).reshape(NCH, 128).T
    )

    # W1 augmented: rows 0..1026 = W1, row 1027 = b1, pad to 1152
    W1 = np.asarray(W1, np.float32)
    w1a = np.zeros((H, 9 * 128, U1), np.float32)
    w1a[:, : U0 + EGO] = W1
    w1a[:, U0 + EGO] = np.asarray(b1, np.float32)
    w1q = _q8(SW1 * w1a)
    # pair part: [p, (h*4+t)*2+j, o] = w1q[h, (2t+j)*128 + p, o]
    w1p_host = np.ascontiguousarray(
        w1q[:, :U0].reshape(H, 4, 2, 128, U1).transpose(3, 0, 1, 2, 4)
        .reshape(128, H * 4 * 2, U1)
    )
    # ego chunk: [p, h*256 + o] = w1q[h, 1024 + p, o]
    w1e_host = np.ascontiguousarray(
        w1q[:, U0:].reshape(H, 128, U1).transpose(1, 0, 2).reshape(128, H * U1)
    )
    # W2 pairs: [p, h*2+j, d] = 16*W2[h, j*128 + p, d]
    w2p_host = np.ascontiguousarray(
        _q8(SW2 * np.asarray(W2, np.float32))
        .reshape(H, 2, 128, 4)
        .transpose(2, 0, 1, 3)
        .reshape(128, H * 2, 4)
    )
    raw_init_std = np.log(np.exp(INIT_STD) - 1.0).astype(np.float32)
    b2m = np.ascontiguousarray(
        np.asarray(b2, np.float32).T
        + np.array([0, 0, raw_init_std, raw_init_std], np.float32)[:, None]
    )  # [4, H]

    # per-row (tanh-scale, exp-scale) for the epilogue
    epi_host = np.array(
        [
            [1 / MEAN_SCALE, 0.0],
            [1 / MEAN_SCALE, 0.0],
            [0.0, -1.0],
            [0.0, -1.0],
        ],
        np.float32,
    )

    in_maps = []
    for c in range(NCORES):
        p = perms[c]
        # x pairs: [kp*128+q, j, b] = xq[perm[b], kp*256 + j*128 + q]
        xp_host = np.ascontiguousarray(
            xq[p].T.reshape(KO, 8, 128, BC).transpose(0, 2, 1, 3)
            .reshape(KO * 128, 8, BC)
        )
        cmd_c = command[p]
        hid_c = np.ascontiguousarray(
            np.broadcast_to(cmd_c[None, :].astype(np.float32), (4, BC)).copy()
        )
        in_maps.append(
            {
                "xp": xp_host,
                "wp": wp_host,
                "bfcT": bfcT,
                "egoT": np.ascontiguousarray(
                    _q8(np.concatenate(
                        [SH * ego_state[p].T, np.full((1, BC), SH, np.float32)], 0
                    ))
                ),
                "w1pd": w1p_host,
                "w1ed": w1e_host,
                "w2pd": w2p_host,
                "b2m": b2m,
                "hid": hid_c,
                "epi": epi_host,
            }
        )
    return in_maps


def run(inputs, trace=False):
    """Build, run on 8 cores; returns (full output [4096,4] f32, results)."""
    from concourse.bass_utils import run_bass_kernel_spmd

    perms, seg = _route(inputs["command"])
    in_maps = _prep_inputs(**inputs, perms=perms)
    nc = _build_graph(seg)
    res = run_bass_kernel_spmd(nc, in_maps, core_ids=list(range(NCORES)), trace=trace)
    full = np.empty((B, 4), np.float32)
    for c in range(NCORES):
        full[perms[c]] = res.results[c]["out"].T
    return full, res


def kernel(**inputs):
    out, _ = run(inputs, trace=False)
    return out


# revision 34
# speedup vs baseline: 1.1815x; 1.1815x over previous
"""Trainium2 Bass kernel for nn_ActionDecoder (moe_routing) — fp8 DoubleRow.

Data-parallel across 8 NeuronCores: batch 4096 -> 512 per core, weights
replicated. Host deals samples to cores balanced per command value and sorts
each core's 512 samples by command, so each head's samples occupy a fixed
column segment [a_h, e_h) (identical offsets on all cores -> one SPMD graph).

All GEMMs run in fp8-e4m3 DoubleRow perf mode with f32 PSUM accumulation;
rel err ~1.7e-3 vs the f32 reference. Power-of-two scales keep descale
exact: x as-is, W_fc*32 -> hp stores 32*h' (range <= 178 < 240 TRN-fp8
max), W1*32 with b1 riding an augmented constant-32 row of the ego chunk,
z1 stores 16*relu(.), W2*16 -> z2 = psum/256 + b2.

Trace-driven notes (HW, per-core; ~85 us vs the 99 us it started at):
- Steady trunk DR matmul = 216 ns (512 cols @ 2.4 GHz, 157 TF/s fp8 peak).
  Pure trunk compute = 57 us; the rest is start (~4.5), head tail (~7),
  and a fixed ~10 us framework exit choreography (drains + sem zeroing).
- DMA: only gpsimd (SWDGE) + scalar/sync (HWDGE) rings exist; ~280-300
  GB/s combined on 4KB lines. Small-packet side-loads steal DMA-engine
  slots from the bulk rings (the old 1KB w1 trickle on sync caused a 6 us
  mid-trunk stall). Now: sync carries oct0's first x slices + ~30KB
  consts + late wq1 quads (octs 3/5/7); x/wq alternate on the two fast
  rings; w1 rides the scalar ring's FIFO tail after all W quads.
- SBUF tile slots pack at 32B alignment; a 32B-offset matmul operand
  halves the PE fetch rate, so every small tile is padded to 64B slots.
- The PE clock ramps only under sustained dense activity, and the
  governor inserts a multi-us half-rate correction after sparse bursts:
  warmup matmuls run from body start until real data lands, and oct0 is
  ordered j-major (8 n-chunks per arrived slice) so the early trunk never
  idles. That ordering alone removed the early throttle window entirely.
- Head phase is woven into the last trunk oct: each hp pair's z1 matmuls
  (all heads, merged [128,2,C] psum per head) are emitted one n-chunk
  after the pair's drains, so they run while trunk work keeps the clock
  high; the final n7 drain splits across ACT+DVE. After the trunk: one
  drain per head, 2 plain z2 matmuls (narrow DR stationary is an ISA
  violation), masked write into one of two parity accumulators, epilogue
  in 3 pipelined column slices with output posts spread over 3 rings.
"""

import numpy as np
import ml_dtypes

B = 4096
EMBED = 8192
U0 = 1024
U1 = 256
EGO = 3
H = 6
NCORES = 8
BC = B // NCORES          # 512 batch per core
KP = EMBED // 256         # 32 trunk k-pairs (DoubleRow: 256 k per matmul)
KO = EMBED // 1024        # 8 trunk k-octs (one x DMA each, 4KB lines)
NCH = U0 // 128           # 8 trunk n-chunks
NWARM = 7                 # 512-col junk matmuls to ramp the PE clock
MEAN_SCALE = 5.0
INIT_STD = 5.0
MIN_STD = 1e-4
SH = 32.0                 # scale on W_fc / hp
SW1 = 32.0                # scale on W1 (+bias row)
SZ = 16.0                 # scale on stored z1
SW2 = 16.0                # scale on W2

FP8 = ml_dtypes.float8_e4m3   # TRN fp8_e4m3 semantics (max 240)


def _build_graph(seg):
    """seg: list of (a_h, C_h) column segments per head, identical on all cores."""
    import concourse.mybir as mybir
    import concourse.tile as tile
    from concourse import bacc

    dt = mybir.dt
    AF = mybir.ActivationFunctionType
    DR = mybir.MatmulPerfMode.DoubleRow

    nc = bacc.Bacc("TRN2", target_bir_lowering=False, debug=False)

    xp = nc.dram_tensor("xp", [KO * 128, 8, BC], dt.float8e4, kind="ExternalInput")
    wp = nc.dram_tensor("wp", [KP * 64, 4, U0], dt.float8e4, kind="ExternalInput")
    bfcT = nc.dram_tensor("bfcT", [128, NCH], dt.float32, kind="ExternalInput")
    egoT = nc.dram_tensor("egoT", [EGO + 1, BC], dt.float8e4, kind="ExternalInput")
    w1pd = nc.dram_tensor("w1pd", [128, H * 4 * 2, 256], dt.float8e4, kind="ExternalInput")
    w1ed = nc.dram_tensor("w1ed", [128, H * 256], dt.float8e4, kind="ExternalInput")
    w2pd = nc.dram_tensor("w2pd", [128, H * 2, 4], dt.float8e4, kind="ExternalInput")
    b2m = nc.dram_tensor("b2m", [4, H], dt.float32, kind="ExternalInput")
    hid = nc.dram_tensor("hid", [4, BC], dt.float32, kind="ExternalInput")
    epi = nc.dram_tensor("epi", [4, 2], dt.float32, kind="ExternalInput")
    out_d = nc.dram_tensor("out", [4, BC], dt.float32, kind="ExternalOutput")

    with tile.TileContext(nc) as tc:
        with (
            tc.tile_pool(name="const", bufs=1) as const,
            tc.tile_pool(name="xk", bufs=8) as xpool,
            tc.tile_pool(name="wk", bufs=16) as wpool,
            tc.tile_pool(name="hp", bufs=1) as hpool,
            tc.tile_pool(name="z1", bufs=1) as zpool,
            tc.tile_pool(name="ps", bufs=8, space="PSUM") as psum,
        ):
            # h' ego chunk: rows 0-2 = 32*ego, row 3 = 32 (bias row), rest 0.
            # memset first on vector so the warmup matmuls can start at body
            # entry, while the DMA rings ramp up.
            hpe = hpool.tile([128, BC], dt.float8e4, tag="hpe")
            nc.vector.memset(hpe[:], 0.0)

            ps_h = [
                psum.tile([128, BC], dt.float32, tag="ps", name=f"ps_h{n}")
                for n in range(NCH)
            ]

            # PE clock warmup: long junk matmuls keep the PE activity monitor
            # busy from body start so the clock ramps to 2.4 GHz during the
            # DMA fill instead of midway through the trunk.
            for i in range(NWARM):
                nc.tensor.matmul(
                    ps_h[NCH - 1][:],
                    hpe[:, 0:128],
                    hpe[:, 0:BC],
                    start=True,
                    stop=True,
                )

            # oct0's x j0/j1 ride the sync HWDGE ring ahead of the consts:
            # sync starts fastest and is otherwise idle, so the first trunk
            # matmul's inputs (64KB x + 128KB W) land earliest.
            xk0 = xpool.tile([128, 8, BC], dt.float8e4, tag="xk", name="xo0")
            nc.sync.dma_start(out=xk0[:, 0, :], in_=xp[0:128, 0, :])
            nc.sync.dma_start(out=xk0[:, 1, :], in_=xp[0:128, 1, :])

            # consts on the sync ring: ~30KB total, needed only late. All
            # slots padded to 64B multiples: SBUF packing aligns only to
            # 32B, and a 32B-offset matmul operand halves the PE fetch rate.
            bfc_t = const.tile([128, NCH], dt.float32, tag="bfc",
                               padded_shape=[128, 16])
            nc.sync.dma_start(out=bfc_t[:], in_=bfcT[:])
            b2_t = const.tile([4, H], dt.float32, tag="b2",
                              padded_shape=[4, 16])
            nc.sync.dma_start(out=b2_t[:], in_=b2m[:])
            hid_t = const.tile([4, BC], dt.float32, tag="hid")
            nc.sync.dma_start(out=hid_t[:], in_=hid[:])
            w2_t = const.tile([128, H * 2, 4], dt.float8e4, tag="w2",
                              padded_shape=[128, 16, 4])
            nc.sync.dma_start(out=w2_t[:], in_=w2pd[:])
            epi_t = const.tile([4, 2], dt.float32, tag="epi",
                               padded_shape=[4, 16])
            nc.sync.dma_start(out=epi_t[:], in_=epi[:])
            nc.sync.dma_start(out=hpe[0 : EGO + 1, :], in_=egoT[:])

            w1p_t = const.tile([128, H * 4 * 2, 256], dt.float8e4, tag="w1p")
            w1e_t = const.tile([128, H * 256], dt.float8e4, tag="w1e")

            hp_pair = [
                hpool.tile([128, 2, BC], dt.float8e4, tag=f"hpp{t}", name=f"hpp{t}")
                for t in range(4)
            ]

            # masked-z2 accumulators: even heads write zA slices, odd heads
            # zB; segments of same-parity heads are disjoint, so each is a
            # plain masked write and the epilogue adds the two.
            zA = const.tile([4, BC], dt.float32, tag="zA")
            zB = const.tile([4, BC], dt.float32, tag="zB")
            nc.vector.memset(zA[:], 0.0)
            nc.vector.memset(zB[:], 0.0)

            # per-head one-hot masks: emitted up front so the vector engine
            # computes them as soon as hid lands (~14us), not at head time
            mks = []
            for h in range(H):
                a, C = seg[h]
                Cp = (C + 15) // 16 * 16
                mk = const.tile([4, C], dt.float32, tag=f"mk_{h}", name=f"mk_{h}",
                                padded_shape=[4, Cp])
                nc.vector.tensor_scalar(
                    mk[:], hid_t[:, a : a + C], float(h + 1), None,
                    mybir.AluOpType.is_equal,
                )
                mks.append(mk)

            # head-phase z1 state, created lazily as the weave needs it
            pzs = [None] * H
            z1ps = [None] * H

            def _z1_alloc(h):
                a, C = seg[h]
                Cp = (C + 31) // 32 * 32
                pzs[h] = psum.tile(
                    [128, 2, C], dt.float32, tag="ps", name=f"pz_{h}"
                )
                z1ps[h] = zpool.tile(
                    [128, 2, C], dt.float8e4, tag=f"z1_{h}", name=f"z1_{h}",
                    padded_shape=[128, 2, Cp],
                )

            def _z1_ego(h):
                a, C = seg[h]
                for m in range(2):
                    nc.tensor.matmul(
                        pzs[h][:, m, :],
                        w1e_t[:, h * 256 + m * 128 : h * 256 + (m + 1) * 128],
                        hpe[:, a : a + C],
                        start=True,
                        stop=False,
                    )

            def _z1_pair(h, t):
                a, C = seg[h]
                g = (h * 4 + t) * 2
                for m in range(2):
                    nc.tensor.matmul(
                        pzs[h][:, m, :],
                        w1p_t[:, g : g + 2, m * 128 : (m + 1) * 128],
                        hp_pair[t][:, :, a : a + C],
                        start=False,
                        stop=(t == 3),
                        perf_mode=DR,
                    )

            # trunk: hp = relu((32Wfc).T @ x + 32 b_fc), DoubleRow over
            # k-pairs; x streams in k-octs and W in k-quads (512KB DMAs with
            # 4KB partition lines; the rings are packet-rate-bound).
            for ob in range(KO):
                first_block = ob == 0
                last_block = ob == KO - 1
                if first_block:
                    xk = xk0
                else:
                    xk = xpool.tile(
                        [128, 8, BC], dt.float8e4, tag="xk", name=f"xo{ob}"
                    )
                wq0 = wpool.tile([128, 4, U0], dt.float8e4, tag="wk", name=f"wq{2 * ob}")
                wq1 = wpool.tile([128, 4, U0], dt.float8e4, tag="wk", name=f"wq{2 * ob + 1}")
                r = slice(ob * 128, (ob + 1) * 128)
                r0 = slice(2 * ob * 128, (2 * ob + 1) * 128)
                r1 = slice((2 * ob + 1) * 128, (2 * ob + 2) * 128)
                if first_block:
                    # ramp: x j0/j1 already posted on sync; W pair-sliced on
                    # scalar, the rest of x + wq1 on gpsimd. Keeping oct0's W
                    # on the fast rings matters doubly: sparse early matmul
                    # activity delays the PE clock ramp and lengthens the
                    # governor's corrective throttle window.
                    nc.scalar.dma_start(out=wq0[:, 0, :], in_=wp[r0, 0, :])
                    nc.scalar.dma_start(out=wq0[:, 1, :], in_=wp[r0, 1, :])
                    nc.gpsimd.dma_start(out=xk[:, 2:4, :], in_=xp[r, 2:4, :])
                    nc.scalar.dma_start(out=wq0[:, 2:4, :], in_=wp[r0, 2:4, :])
                    nc.gpsimd.dma_start(out=xk[:, 4:8, :], in_=xp[r, 4:8, :])
                    nc.gpsimd.dma_start(out=wq1[:, 0:2, :], in_=wp[r1, 0:2, :])
                    nc.gpsimd.dma_start(out=wq1[:, 2:4, :], in_=wp[r1, 2:4, :])
                else:
                    # x + wq0 alternate over the two fast rings; sync (the
                    # slowest ring, ~0.5MB/10us) carries only the late wq1
                    # quads of octs 3/5/7, each due well after it can land.
                    qa, qb_ = (nc.scalar, nc.gpsimd) if ob % 2 else (nc.gpsimd, nc.scalar)
                    qa.dma_start(out=xk[:], in_=xp[r])
                    qb_.dma_start(out=wq0[:], in_=wp[r0])
                    if ob % 2 and ob > 1:
                        nc.sync.dma_start(out=wq1[:], in_=wp[r1])
                    else:
                        qa.dma_start(out=wq1[:], in_=wp[r1])
                if last_block:
                    # w1 rides the scalar ring's FIFO tail: delivered after
                    # every W quad (~50us), needed by the z1 weave (~70us).
                    nc.scalar.dma_start(out=w1p_t[:, :24, :], in_=w1pd[:, :24, :])
                    nc.scalar.dma_start(out=w1p_t[:, 24:, :], in_=w1pd[:, 24:, :])
                    nc.scalar.dma_start(out=w1e_t[:], in_=w1ed[:])
                if first_block:
                    # j-major: a full row of 8 n-chunk matmuls per j-slice,
                    # so each row's ~3.5us of PE work covers the DMA ramp
                    # latency of the next slice and the PE never stalls on a
                    # W slice that hasn't landed yet.
                    for j in range(2):
                        for n in range(NCH):
                            nc.tensor.matmul(
                                ps_h[n][:],
                                wq0[:, j, n * 128 : (n + 1) * 128],
                                xk[:, j, :],
                                start=(j == 0),
                                stop=False,
                            )
                    for pp in range(1, 4):
                        wq, u = (wq0, 1) if pp < 2 else (wq1, pp - 2)
                        for n in range(NCH):
                            nc.tensor.matmul(
                                ps_h[n][:],
                                wq[:, 2 * u : 2 * u + 2, n * 128 : (n + 1) * 128],
                                xk[:, 2 * pp : 2 * pp + 2, :],
                                start=False,
                                stop=False,
                                perf_mode=DR,
                            )
                    continue
                for n in range(NCH):
                    if True:
                        for pp in range(4):
                            wq, u = (wq0, pp) if pp < 2 else (wq1, pp - 2)
                            nc.tensor.matmul(
                                ps_h[n][:],
                                wq[:, 2 * u : 2 * u + 2, n * 128 : (n + 1) * 128],
                                xk[:, 2 * pp : 2 * pp + 2, :],
                                start=False,
                                stop=(last_block and pp == 3),
                                perf_mode=DR,
                            )
                    if last_block:
                        # n-chunk complete: drain to fp8 SBUF (relu + bias).
                        # n7 splits across both engines so the final pair is
                        # ready ~0.45us after the last trunk matmul.
                        tgt = hp_pair[n // 2][:, n % 2, :]
                        if n == 7:
                            nc.scalar.activation(
                                tgt[:, 0:256], ps_h[n][:, 0:256], AF.Relu,
                                bias=bfc_t[:, n : n + 1], scale=1.0,
                            )
                            nc.vector.tensor_scalar(
                                tgt[:, 256:BC], ps_h[n][:, 256:BC],
                                bfc_t[:, n : n + 1], 0.0,
                                mybir.AluOpType.add, mybir.AluOpType.max,
                            )
                        elif n % 2 == 0:
                            nc.scalar.activation(
                                tgt, ps_h[n][:], AF.Relu,
                                bias=bfc_t[:, n : n + 1], scale=1.0,
                            )
                        else:
                            nc.vector.tensor_scalar(
                                tgt, ps_h[n][:], bfc_t[:, n : n + 1], 0.0,
                                mybir.AluOpType.add, mybir.AluOpType.max,
                            )
                        # weave head z1 matmuls into the last oct, each batch
                        # one n-chunk after its pair drains so the drains
                        # finish under trunk matmul cover and the PE never
                        # stalls; the z1 work runs while trunk activity keeps
                        # the PE clock at full rate.
                        if n == 2:
                            for h in (0, 1):
                                _z1_alloc(h)
                                _z1_ego(h)
                                _z1_pair(h, 0)
                        elif n == 4:
                            for h in (2, 3):
                                _z1_alloc(h)
                                _z1_ego(h)
                                _z1_pair(h, 0)
                            for h in range(4):
                                _z1_pair(h, 1)
                        elif n == 6:
                            for h in (4, 5):
                                _z1_alloc(h)
                                _z1_ego(h)
                                _z1_pair(h, 0)
                                _z1_pair(h, 1)
                            for h in range(H):
                                _z1_pair(h, 2)
                        elif n == 7:
                            for h in range(H):
                                _z1_pair(h, 3)

            s4 = const.tile([4, BC], dt.float32, tag="s4")
            u4 = const.tile([4, BC], dt.float32, tag="u4")
            a4 = const.tile([4, BC], dt.float32, tag="a4")
            t4 = const.tile([4, BC], dt.float32, tag="t4")
            m4 = const.tile([4, BC], dt.float32, tag="m4")

            def _emit_epilogue(sl, mean_ring=None):
                # mean rows 0-1: 5*tanh(z/5); std rows 2-3: softplus(z+b) ~=
                # z + exp(-z) (z ~ 5 here). ACT/DVE can't start at a nonzero
                # partition, so both row pairs run over all 4 partitions with
                # per-row scales (junk in the other pair); the output DMAs
                # are row-disjoint and ride two rings to halve post latency.
                nc.vector.tensor_tensor(
                    s4[:, sl], zA[:, sl], zB[:, sl], mybir.AluOpType.add
                )
                nc.scalar.activation(
                    u4[:, sl], s4[:, sl], AF.Exp, scale=epi_t[:, 1:2]
                )
                nc.scalar.activation(
                    t4[:, sl], s4[:, sl], AF.Tanh, scale=epi_t[:, 0:1]
                )
                nc.vector.tensor_tensor(
                    a4[:, sl], s4[:, sl], u4[:, sl], mybir.AluOpType.add
                )
                nc.vector.tensor_scalar_mul(m4[:, sl], t4[:, sl], MEAN_SCALE)
                nc.sync.dma_start(out=out_d[2:4, sl], in_=a4[2:4, sl])
                (mean_ring or nc.gpsimd).dma_start(
                    out=out_d[0:2, sl], in_=m4[0:2, sl]
                )

            # z1 drains first (all heads' z1 matmuls already flowed during
            # the last oct), then z2 + masked write per head; epilogue in 3
            # pipelined column slices.
            for h in range(H):
                if h % 2 == 0:
                    nc.scalar.activation(
                        z1ps[h][:], pzs[h][:], AF.Relu, scale=SZ / (SH * SW1)
                    )
                else:
                    nc.vector.tensor_scalar(
                        z1ps[h][:], pzs[h][:], SZ / (SH * SW1), 0.0,
                        mybir.AluOpType.mult, mybir.AluOpType.max,
                    )
            slice_bounds = [seg[3][0], seg[5][0], BC]
            slice_after = {2: 0, 4: 1, 5: 2}
            fin = 0
            for h in range(H):
                a, C = seg[h]
                # z2: narrow stationary [128,2,4] fails the DR fp8 LDW ISA
                # restriction, so two plain matmuls over the m-chunks.
                pz2 = psum.tile([4, C], dt.float32, tag="ps", name=f"pz2_{h}")
                for m in range(2):
                    nc.tensor.matmul(
                        pz2[:],
                        w2_t[:, h * 2 + m, :],
                        z1ps[h][:, m, :],
                        start=(m == 0),
                        stop=(m == 1),
                    )
                z2s = const.tile([4, C], dt.float32, tag=f"z2s_{h}", name=f"z2s_{h}",
                                 padded_shape=[4, (C + 15) // 16 * 16])
                if h % 2 == 0:
                    nc.vector.tensor_scalar(
                        z2s[:], pz2[:], 1.0 / (SZ * SW2), b2_t[:, h : h + 1],
                        mybir.AluOpType.mult, mybir.AluOpType.add,
                    )
                else:
                    nc.scalar.activation(
                        z2s[:], pz2[:], AF.Identity,
                        bias=b2_t[:, h : h + 1], scale=1.0 / (SZ * SW2),
                    )
                zX = zA if h % 2 == 0 else zB
                nc.vector.tensor_tensor(
                    zX[:, a : a + C], z2s[:], mks[h][:], mybir.AluOpType.mult
                )
                if h in slice_after:
                    si = slice_after[h]
                    end = slice_bounds[si]
                    if end > fin:
                        # the last slice's mean post rides the scalar ring,
                        # idle once its activations are done
                        _emit_epilogue(
                            slice(fin, end),
                            mean_ring=nc.scalar if si == 2 else None,
                        )
                        fin = end

    nc.compile()
    return nc


def _route(command):
    """Deal samples to cores balanced per head; sort each core by head.

    Returns (perms, seg): perms[c] = global sample indices for core c in
    column order; seg[h] = (a_h, C_h) identical across cores, covering every
    head-h sample's column on every core.
    """
    command = np.asarray(command, dtype=np.int32)
    glob_counts = np.array([(command == h + 1).sum() for h in range(H)], np.int64)
    shares = np.tile(glob_counts // NCORES, (NCORES, 1))
    ptr = 0
    for h in range(H):
        for _ in range(int(glob_counts[h] % NCORES)):
            shares[ptr % NCORES, h] += 1
            ptr += 1
    assert (shares.sum(axis=1) == BC).all()
    percore = [[] for _ in range(NCORES)]
    counts = np.zeros((NCORES, H), np.int64)
    for h in range(H):
        idx = np.nonzero(command == h + 1)[0]
        off = 0
        for c in range(NCORES):
            share = idx[off : off + shares[c, h]]
            off += shares[c, h]
            percore[c].append(share)
            counts[c, h] = len(share)
    perms = [np.concatenate(percore[c]) for c in range(NCORES)]
    starts = np.zeros((NCORES, H), np.int64)
    starts[:, 1:] = np.cumsum(counts, axis=1)[:, :-1]
    ends = starts + counts
    seg = []
    for h in range(H):
        a = int(starts[:, h].min())
        e = int(ends[:, h].max())
        if e <= a:  # head empty on every core: 1 masked-off dummy column
            a = min(a, BC - 1)
            e = a + 1
        seg.append((a, e - a))
    for c in range(NCORES):
        for h in range(H):
            a, C = seg[h]
            assert starts[c, h] >= a and ends[c, h] <= a + C
        assert len(perms[c]) == BC
    # same-parity heads must have disjoint segments (zA/zB write scheme)
    for h in range(H - 2):
        assert seg[h][0] + seg[h][1] <= seg[h + 2][0]
    return perms, seg


def _q8(a):
    return np.clip(a, -240.0, 240.0).astype(FP8)


def _prep_inputs(x, command, ego_state, W_fc, b_fc, W1, b1, W2, b2, perms):
    """Host-side shard + layout prep. Returns in_maps for 8 cores."""
    x = np.asarray(x, dtype=np.float32)
    command = np.asarray(command, dtype=np.int32)
    ego_state = np.asarray(ego_state, dtype=np.float32)

    xq = _q8(x)  # [B, EMBED] fp8
    # W_fc pairs: [kp*128+p, j, n] = 32*Wfc[kp*256 + j*128 + p, n]
    wp_host = np.ascontiguousarray(
        _q8(SH * np.asarray(W_fc, np.float32))
        .reshape(KP // 2, 4, 128, U0)
        .transpose(0, 2, 1, 3)
        .reshape(KP * 64, 4, U0)
    )
    bfcT = np.ascontiguousarray(
        (SH * np.asarray(b_fc, np.float32)).reshape(NCH, 128).T
    )

    # W1 augmented: rows 0..1026 = W1, row 1027 = b1, pad to 1152
    W1 = np.asarray(W1, np.float32)
    w1a = np.zeros((H, 9 * 128, U1), np.float32)
    w1a[:, : U0 + EGO] = W1
    w1a[:, U0 + EGO] = np.asarray(b1, np.float32)
    w1q = _q8(SW1 * w1a)
    # pair part: [p, (h*4+t)*2+j, o] = w1q[h, (2t+j)*128 + p, o]
    w1p_host = np.ascontiguousarray(
        w1q[:, :U0].reshape(H, 4, 2, 128, U1).transpose(3, 0, 1, 2, 4)
        .reshape(128, H * 4 * 2, U1)
    )
    # ego chunk: [p, h*256 + o] = w1q[h, 1024 + p, o]
    w1e_host = np.ascontiguousarray(
        w1q[:, U0:].reshape(H, 128, U1).transpose(1, 0, 2).reshape(128, H * U1)
    )
    # W2 pairs: [p, h*2+j, d] = 16*W2[h, j*128 + p, d]
    w2p_host = np.ascontiguousarray(
        _q8(SW2 * np.asarray(W2, np.float32))
        .reshape(H, 2, 128, 4)
        .transpose(2, 0, 1, 3)
        .reshape(128, H * 2, 4)
    )
    raw_init_std = np.log(np.exp(INIT_STD) - 1.0).astype(np.float32)
    b2m = np.ascontiguousarray(
        np.asarray(b2, np.float32).T
        + np.array([0, 0, raw_init_std, raw_init_std], np.float32)[:, None]
    )  # [4, H]

    # per-row (tanh-scale, exp-scale) for the epilogue
    epi_host = np.array(
        [
            [1 / MEAN_SCALE, 0.0],
            [1 / MEAN_SCALE, 0.0],
            [0.0, -1.0],
            [0.0, -1.0],
        ],
        np.float32,
    )

    in_maps = []
    for c in range(NCORES):
        p = perms[c]
        # x pairs: [kp*128+q, j, b] = xq[perm[b], kp*256 + j*128 + q]
        xp_host = np.ascontiguousarray(
            xq[p].T.reshape(KO, 8, 128, BC).transpose(0, 2, 1, 3)
            .reshape(KO * 128, 8, BC)
        )
        cmd_c = command[p]
        hid_c = np.ascontiguousarray(
            np.broadcast_to(cmd_c[None, :].astype(np.float32), (4, BC)).copy()
        )
        in_maps.append(
            {
                "xp": xp_host,
                "wp": wp_host,
                "bfcT": bfcT,
                "egoT": np.ascontiguousarray(
                    _q8(np.concatenate(
                        [SH * ego_state[p].T, np.full((1, BC), SH, np.float32)], 0
                    ))
                ),
                "w1pd": w1p_host,
                "w1ed": w1e_host,
                "w2pd": w2p_host,
                "b2m": b2m,
                "hid": hid_c,
                "epi": epi_host,
            }
        )
    return in_maps


def run(inputs, trace=False):
    """Build, run on 8 cores; returns (full output [4096,4] f32, results)."""
    from concourse.bass_utils import run_bass_kernel_spmd

    perms, seg = _route(inputs["command"])
    in_maps = _prep_inputs(**inputs, perms=perms)
    nc = _build_graph(seg)
    res = run_bass_kernel_spmd(nc, in_maps, core_ids=list(range(NCORES)), trace=trace)
    full = np.empty((B, 4), np.float32)
    for c in range(NCORES):
        full[perms[c]] = res.results[c]["out"].T
    return full, res


def kernel(**inputs):
    out, _ = run(inputs, trace=False)
    return out
